# revision 23
# baseline (speedup 1.0000x reference)
"""Trainium2 Bass kernel for nn_Encoder_61770219651232 (dual-quaternion skinning).

Computation per node n (N = 2,000,000):
    qs = W[n, :10] @ qm4            (qm4 = x.reshape(10, 4), shared)
    q  = qs / |qs|                  (normalize)
    y3 = R(q) @ v                   (rotate v = VR[4n:4n+3])
    y  = [y3, r]                    (r = VR[4n+3] passes through)

Strategy (pure data parallel over nodes, 8 cores, all fp32):
  - W loads contiguously as (128, 1920) tiles; each 120-float column chunk
    holds 12 nodes x 10 weights (a "dozen").
  - PE transpose #1: (128, 120) slice -> (120, 128): puts the (node-in-dozen,
    weight) flat offset on partitions, dozens on the free axis.
  - Fused blend+transpose matmul: qt_c = Wt_slice.T @ blockdiag(qm4):
    stationary = a (120, 128) Wt slice, moving = the (120, 48) block-diagonal
    qm4. One matmul both applies qm4 and lands quaternions NODE-MAJOR
    interleaved (128 partitions x [qx qy qz qw] runs) -- exactly matching a
    naturally-loaded VR tile, so no further data movement is needed.
  - DVE/ACT rotation with unnormalized q (no sqrt):
        y3 = v + (2/|q|^2) * (qw*(qv x v) + qv x (qv x v))
    ACT does squares (scale=1/sqrt(2) folds the 2) and PSUM->SBUF copies;
    DVE does the cross products (scalar_tensor_tensor FMAs) and
    reciprocal_approx_fast (2/|q|^2 without sqrt, ~51 ULP).
  - y written in-place into the VR tile, contiguous DMA out.
Scale-relative error vs the fp32 jax reference: ~5e-6.
Cost-model (TimelineSim) estimate: ~103 us/core; DVE-bound (~94% DVE
occupancy; cross-product stages packed into fat 3-component tiles; W loads
issued on the scalar-engine HWDGE ring, VR/y on sync, to split DMA issue).
"""
import sys

sys.path.insert(0, "/opt/trn_rl_repo")

import numpy as np

N_NODES = 2_000_000
N_CORES = 8
MB_NODES = 24576          # nodes per megablock (2048 dozens)
NMB = 11                  # megablocks per core
NPC = MB_NODES * NMB      # 270336 nodes per core
N_PAD = NPC * N_CORES     # 2162688 padded total
GRANULES = [(0, 1), (1, 2), (3, 2), (5, 2), (7, 2), (9, 2)]  # (first mb, num mbs) rotate granules

# "f32" = exact fp32 matmuls (4 cyc/row); "f32r" = single-pass PE mode
# (1-1.5 cyc/row); precision measured empirically in test.py.
MM_MODE = "f32"

_compiled = None


def _build_kernel():
    import concourse.bacc as bacc
    import concourse.tile as tile
    from concourse import mybir

    f32 = mybir.dt.float32
    Alu = mybir.AluOpType
    Act = mybir.ActivationFunctionType

    nc = bacc.Bacc("TRN2", target_bir_lowering=False, debug=False,
                   num_devices=N_CORES)

    w_dram = nc.dram_tensor("w", [NPC * 10], f32, kind="ExternalInput")
    vr_dram = nc.dram_tensor("vr", [NPC * 4], f32, kind="ExternalInput")
    bd_dram = nc.dram_tensor("bd", [120, 48], f32, kind="ExternalInput")
    id_dram = nc.dram_tensor("ident", [128, 128], f32, kind="ExternalInput")
    y_dram = nc.dram_tensor("y", [NPC * 4], f32, kind="ExternalOutput")

    w3 = w_dram.ap().rearrange("(m p e) -> m p e", m=NMB, p=128)      # e=1920
    vr3 = vr_dram.ap().rearrange("(m f e) -> m f e", m=NMB, f=128)    # e=768
    y3 = y_dram.ap().rearrange("(m f e) -> m f e", m=NMB, f=128)

    from contextlib import ExitStack

    with tile.TileContext(nc) as tc, ExitStack() as ctx:
        consts = ctx.enter_context(tc.tile_pool(name="consts", bufs=1))
        wpool = ctx.enter_context(tc.tile_pool(name="wpool", bufs=3))
        wtpool = ctx.enter_context(tc.tile_pool(name="wtpool", bufs=2))
        gran_pool = ctx.enter_context(tc.tile_pool(name="gran", bufs=3))
        scratch = ctx.enter_context(tc.tile_pool(name="scratch", bufs=2))
        wt_psp = ctx.enter_context(tc.tile_pool(name="wt_ps", bufs=2, space="PSUM"))
        qt_psp = ctx.enter_context(tc.tile_pool(name="qt_ps", bufs=2, space="PSUM"))

        bd_sb = consts.tile([120, 48], f32)
        nc.sync.dma_start(out=bd_sb[:], in_=bd_dram.ap())
        id_sb = consts.tile([128, 128], f32)
        nc.sync.dma_start(out=id_sb[:], in_=id_dram.ap())

        def mmv(ap):
            """matmul-operand view, optionally bitcast to float32r"""
            return ap.bitcast(mybir.dt.float32r) if MM_MODE == "f32r" else ap

        for g0, gn in GRANULES:
            fd = 768 * gn            # interleaved free size for this granule
            n_el = fd // 4           # per-component element count
            qt_gran = gran_pool.tile([128, fd], f32, tag="qt_gran")
            vr_gran = gran_pool.tile([128, fd], f32, tag="vr_gran")

            for k in range(gn):
                mb = g0 + k
                # ---- load W megablock + VR slice ----
                w_big = wpool.tile([128, 1920], f32, tag="w_big")
                nc.sync.dma_start(out=w_big[:], in_=w3[mb])
                nc.sync.dma_start(out=vr_gran[:, 768 * k:768 * (k + 1)],
                                  in_=vr3[mb])
                # ---- T1: 16 PE transposes -> wt_sb (120, 2048) ----
                wt_sb = wtpool.tile([120, 2048], f32, tag="wt_sb")
                for b in range(4):
                    wt_ps = wt_psp.tile([120, 512], f32, tag="wt_ps")
                    for t4 in range(4):
                        t = 4 * b + t4
                        nc.tensor.transpose(
                            mmv(wt_ps[:, 128 * t4:128 * (t4 + 1)]),
                            mmv(w_big[:, 120 * t:120 * (t + 1)]),
                            mmv(id_sb[:]),
                        )
                    nc.scalar.copy(out=wt_sb[:, 512 * b:512 * (b + 1)],
                                   in_=wt_ps[:])
                # ---- fused blend+transpose: qt_c = Wt_slice.T @ BD ----
                # out[f, 4s+j] = sum_k Wt[k, 128c+f] * BD[k, 4s+j]
                #             = qs_j(node 12*(16f+c)+s): node-major interleaved
                for bank in range(2):
                    qt_ps = qt_psp.tile([128, 384], f32, tag="qt_ps")
                    for cc in range(8):
                        c = 8 * bank + cc
                        nc.tensor.matmul(
                            qt_ps[:, 48 * cc:48 * (cc + 1)],
                            mmv(wt_sb[:, 128 * c:128 * (c + 1)]),
                            mmv(bd_sb[:]),
                        )
                    off = 768 * k + 384 * bank
                    nc.scalar.copy(out=qt_gran[:, off:off + 384], in_=qt_ps[:])

            # ---- rotate on the whole granule ----
            Q = qt_gran[:, :fd].rearrange("p (n e) -> p n e", e=4)
            V = vr_gran[:, :fd].rearrange("p (n e) -> p n e", e=4)
            qx, qy, qz, qw = (Q[:, :, i:i + 1] for i in range(4))
            vx, vy, vz = (V[:, :, i:i + 1] for i in range(3))

            def st(tag, width=1):
                return scratch.tile([128, n_el, width], f32, tag=tag, name=tag)

            # |q|^2/2 via ACT squares with scale 1/sqrt(2), tree-added fat
            isq = float(np.sqrt(0.5))
            sqp = st("sqp", 4)
            for i, qc in enumerate((qx, qy, qz, qw)):
                nc.scalar.activation(sqp[:, :, i:i + 1], qc, Act.Square, scale=isq)
            s2 = st("s2", 2)
            nc.vector.tensor_add(s2[:], sqp[:, :, 0:2], sqp[:, :, 2:4])
            n2h = st("n2h")
            nc.vector.tensor_add(n2h[:], s2[:, :, 0:1], s2[:, :, 1:2])
            gg = st("gg")
            # 2/|q|^2 = exp(-log(|q|^2/2)) on ACT (frees DVE; log+exp share
            # one table set)
            nc.scalar.activation(gg[:], n2h[:], Act.Ln)
            nc.scalar.activation(gg[:], gg[:], Act.Exp, scale=-1.0)

            # t = qv x v (into fat tile T): fat products then one fat sub
            T = st("T", 3)
            C = st("C", 3)
            P = st("P", 3)
            Qm = st("Qm", 3)
            for (i, (a1, b1), (a2, b2)) in (
                (0, (qy, vz), (qz, vy)),
                (1, (qz, vx), (qx, vz)),
                (2, (qx, vy), (qy, vx)),
            ):
                nc.vector.tensor_mul(P[:, :, i:i + 1], a1, b1)
                nc.vector.tensor_mul(Qm[:, :, i:i + 1], a2, b2)
            nc.vector.scalar_tensor_tensor(
                out=T[:], in0=Qm[:], scalar=-1.0, in1=P[:],
                op0=Alu.mult, op1=Alu.add)

            # c = qv x t (into C), wt = qw*t (into WT)
            WT = st("WT", 3)
            tv = [T[:, :, i:i + 1] for i in range(3)]
            for i in range(3):
                nc.vector.tensor_mul(WT[:, :, i:i + 1], qw, tv[i])
            for (i, (a1, b1), (a2, b2)) in (
                (0, (qy, tv[2]), (qz, tv[1])),
                (1, (qz, tv[0]), (qx, tv[2])),
                (2, (qx, tv[1]), (qy, tv[0])),
            ):
                nc.vector.tensor_mul(P[:, :, i:i + 1], a1, b1)
                nc.vector.tensor_mul(Qm[:, :, i:i + 1], a2, b2)
            nc.vector.scalar_tensor_tensor(
                out=C[:], in0=Qm[:], scalar=-1.0, in1=P[:],
                op0=Alu.mult, op1=Alu.add)

            # m = c + wt (fat); e = m*g; y = v + e (fat)
            nc.vector.tensor_add(C[:], C[:], WT[:])
            for i in range(3):
                nc.vector.tensor_mul(C[:, :, i:i + 1], C[:, :, i:i + 1], gg[:])
            nc.vector.tensor_add(V[:, :, 0:3], C[:], V[:, :, 0:3])

            # ---- store y (in-place in vr_gran) ----
            for k in range(gn):
                nc.sync.dma_start(out=y3[g0 + k],
                                  in_=vr_gran[:, 768 * k:768 * (k + 1)])

    nc.compile()
    return nc


def _get_compiled():
    global _compiled
    if _compiled is None:
        _compiled = _build_kernel()
    return _compiled


def kernel(x, weights, VR):
    from concourse import bass_utils

    x = np.asarray(x, dtype=np.float32)
    weights = np.asarray(weights, dtype=np.float32)
    VR = np.asarray(VR, dtype=np.float32)

    qm4 = x.reshape(10, 4)
    bd = np.zeros((120, 48), np.float32)
    for s in range(12):
        bd[10 * s:10 * s + 10, 4 * s:4 * s + 4] = qm4
    ident = np.eye(128, dtype=np.float32)

    w_pad = np.zeros((N_PAD, 10), np.float32)
    w_pad[:N_NODES] = weights
    vr_pad = np.zeros(N_PAD * 4, np.float32)
    vr_pad[:N_NODES * 4] = VR

    w_shards = w_pad.reshape(N_CORES, NPC * 10)
    vr_shards = vr_pad.reshape(N_CORES, NPC * 4)

    nc = _get_compiled()
    in_maps = [
        {"w": w_shards[c], "vr": vr_shards[c], "bd": bd, "ident": ident}
        for c in range(N_CORES)
    ]
    res = bass_utils.run_bass_kernel_spmd(nc, in_maps, core_ids=list(range(N_CORES)))
    y = np.concatenate([res.results[c]["y"] for c in range(N_CORES)])
    return y[:N_NODES * 4].astype(np.float32)


if __name__ == "__main__":
    # quick self-check with random data
    rng = np.random.default_rng(0)
    x = rng.standard_normal(40).astype(np.float32)
    W = (rng.standard_normal((N_NODES, 10)) * 0.1).astype(np.float32)
    VR = rng.standard_normal(N_NODES * 4).astype(np.float32)
    y = kernel(x, weights=W, VR=VR)
    print("kernel ran, y shape", y.shape, y[:8])



# revision 28
# speedup vs baseline: 1.0087x; 1.0087x over previous
"""Trainium2 Bass kernel for nn_Encoder_61770219651232 (dual-quaternion skinning).

Computation per node n (N = 2,000,000):
    qs = W[n, :10] @ qm4            (qm4 = x.reshape(10, 4), shared)
    q  = qs / |qs|                  (normalize)
    y3 = R(q) @ v                   (rotate v = VR[4n:4n+3])
    y  = [y3, r]                    (r = VR[4n+3] passes through)

Strategy (pure data parallel over nodes, 8 cores, all fp32):
  - W loads contiguously as (128, 1920) tiles; each 120-float column chunk
    holds 12 nodes x 10 weights (a "dozen").
  - PE transpose #1: (128, 120) slice -> (120, 128): puts the (node-in-dozen,
    weight) flat offset on partitions, dozens on the free axis.
  - Fused blend+transpose matmul: qt_c = Wt_slice.T @ blockdiag(qm4):
    stationary = a (120, 128) Wt slice, moving = the (120, 48) block-diagonal
    qm4. One matmul both applies qm4 and lands quaternions NODE-MAJOR
    interleaved (128 partitions x [qx qy qz qw] runs) -- exactly matching a
    naturally-loaded VR tile, so no further data movement is needed.
  - DVE/ACT rotation with unnormalized q (no sqrt):
        y3 = v + (2/|q|^2) * (qw*(qv x v) + qv x (qv x v))
    ACT does squares (scale=1/sqrt(2) folds the 2) and PSUM->SBUF copies;
    DVE does the cross products (scalar_tensor_tensor FMAs) and
    reciprocal_approx_fast (2/|q|^2 without sqrt, ~51 ULP).
  - y written in-place into the VR tile, contiguous DMA out.
Scale-relative error vs the fp32 jax reference: ~5e-6.
Cost-model (TimelineSim) estimate: ~102 us/core; DVE-bound (~94% DVE
occupancy; cross-product stages packed into fat 3-component tiles; W loads
issued on the scalar-engine HWDGE ring, VR/y on sync, to split DMA issue).
"""
import sys

sys.path.insert(0, "/opt/trn_rl_repo")

import numpy as np

N_NODES = 2_000_000
N_CORES = 8
MB_NODES = 24576          # nodes per megablock (2048 dozens)
NMB = 11                  # megablocks per core
NPC = MB_NODES * NMB      # 270336 nodes per core
N_PAD = NPC * N_CORES     # 2162688 padded total
GRANULES = [(0, 1), (1, 2), (3, 2), (5, 2), (7, 2), (9, 2)]  # (first mb, num mbs) rotate granules

# "f32" = exact fp32 matmuls (4 cyc/row); "f32r" = single-pass PE mode
# (1-1.5 cyc/row); precision measured empirically in test.py.
MM_MODE = "f32"

_compiled = None


def _build_kernel():
    import concourse.bacc as bacc
    import concourse.tile as tile
    from concourse import mybir

    f32 = mybir.dt.float32
    Alu = mybir.AluOpType
    Act = mybir.ActivationFunctionType

    nc = bacc.Bacc("TRN2", target_bir_lowering=False, debug=False,
                   num_devices=N_CORES)

    w_dram = nc.dram_tensor("w", [NPC * 10], f32, kind="ExternalInput")
    vr_dram = nc.dram_tensor("vr", [NPC * 4], f32, kind="ExternalInput")
    bd_dram = nc.dram_tensor("bd", [120, 48], f32, kind="ExternalInput")
    id_dram = nc.dram_tensor("ident", [128, 128], f32, kind="ExternalInput")
    y_dram = nc.dram_tensor("y", [NPC * 4], f32, kind="ExternalOutput")

    w3 = w_dram.ap().rearrange("(m p e) -> m p e", m=NMB, p=128)      # e=1920
    vr3 = vr_dram.ap().rearrange("(m f e) -> m f e", m=NMB, f=128)    # e=768
    y3 = y_dram.ap().rearrange("(m f e) -> m f e", m=NMB, f=128)

    from contextlib import ExitStack

    with tile.TileContext(nc) as tc, ExitStack() as ctx:
        consts = ctx.enter_context(tc.tile_pool(name="consts", bufs=1))
        wpool = ctx.enter_context(tc.tile_pool(name="wpool", bufs=3))
        wtpool = ctx.enter_context(tc.tile_pool(name="wtpool", bufs=2))
        gran_pool = ctx.enter_context(tc.tile_pool(name="gran", bufs=3))
        scratch = ctx.enter_context(tc.tile_pool(name="scratch", bufs=2))
        wt_psp = ctx.enter_context(tc.tile_pool(name="wt_ps", bufs=2, space="PSUM"))
        qt_psp = ctx.enter_context(tc.tile_pool(name="qt_ps", bufs=2, space="PSUM"))

        bd_sb = consts.tile([120, 48], f32)
        nc.sync.dma_start(out=bd_sb[:], in_=bd_dram.ap())
        id_sb = consts.tile([128, 128], f32)
        nc.sync.dma_start(out=id_sb[:], in_=id_dram.ap())

        def mmv(ap):
            """matmul-operand view, optionally bitcast to float32r"""
            return ap.bitcast(mybir.dt.float32r) if MM_MODE == "f32r" else ap

        for g0, gn in GRANULES:
            fd = 768 * gn            # interleaved free size for this granule
            n_el = fd // 4           # per-component element count
            qt_gran = gran_pool.tile([128, fd], f32, tag="qt_gran")
            vr_gran = gran_pool.tile([128, fd], f32, tag="vr_gran")

            for k in range(gn):
                mb = g0 + k
                # ---- load W megablock + VR slice ----
                w_big = wpool.tile([128, 1920], f32, tag="w_big")
                nc.sync.dma_start(out=w_big[:], in_=w3[mb])
                nc.sync.dma_start(out=vr_gran[:, 768 * k:768 * (k + 1)],
                                  in_=vr3[mb])
                # ---- T1: 16 PE transposes -> wt_sb (120, 2048) ----
                wt_sb = wtpool.tile([120, 2048], f32, tag="wt_sb")
                for b in range(4):
                    wt_ps = wt_psp.tile([120, 512], f32, tag="wt_ps")
                    for t4 in range(4):
                        t = 4 * b + t4
                        nc.tensor.transpose(
                            mmv(wt_ps[:, 128 * t4:128 * (t4 + 1)]),
                            mmv(w_big[:, 120 * t:120 * (t + 1)]),
                            mmv(id_sb[:]),
                        )
                    nc.scalar.copy(out=wt_sb[:, 512 * b:512 * (b + 1)],
                                   in_=wt_ps[:])
                # ---- fused blend+transpose: qt_c = Wt_slice.T @ BD ----
                # out[f, 4s+j] = sum_k Wt[k, 128c+f] * BD[k, 4s+j]
                #             = qs_j(node 12*(16f+c)+s): node-major interleaved
                for bank in range(2):
                    qt_ps = qt_psp.tile([128, 384], f32, tag="qt_ps")
                    for cc in range(8):
                        c = 8 * bank + cc
                        nc.tensor.matmul(
                            qt_ps[:, 48 * cc:48 * (cc + 1)],
                            mmv(wt_sb[:, 128 * c:128 * (c + 1)]),
                            mmv(bd_sb[:]),
                        )
                    off = 768 * k + 384 * bank
                    nc.scalar.copy(out=qt_gran[:, off:off + 384], in_=qt_ps[:])

            # ---- rotate on the whole granule ----
            Q = qt_gran[:, :fd].rearrange("p (n e) -> p n e", e=4)
            V = vr_gran[:, :fd].rearrange("p (n e) -> p n e", e=4)
            qx, qy, qz, qw = (Q[:, :, i:i + 1] for i in range(4))
            vx, vy, vz = (V[:, :, i:i + 1] for i in range(3))

            def st(tag, width=1):
                return scratch.tile([128, n_el, width], f32, tag=tag, name=tag)

            # |q|^2/2 via ACT squares with scale 1/sqrt(2), tree-added fat
            isq = float(np.sqrt(0.5))
            sqp = st("sqp", 4)
            for i, qc in enumerate((qx, qy, qz, qw)):
                nc.scalar.activation(sqp[:, :, i:i + 1], qc, Act.Square, scale=isq)
            s2 = st("s2", 2)
            nc.vector.tensor_add(s2[:], sqp[:, :, 0:2], sqp[:, :, 2:4])
            n2h = st("n2h")
            nc.vector.tensor_add(n2h[:], s2[:, :, 0:1], s2[:, :, 1:2])
            gg = st("gg")
            # 2/|q|^2 = exp(-log(|q|^2/2)) on ACT (frees DVE; log+exp share
            # one table set)
            nc.scalar.activation(gg[:], n2h[:], Act.Ln)
            nc.scalar.activation(gg[:], gg[:], Act.Exp, scale=-1.0)

            # t = qv x v (into fat tile T): fat products then one fat sub
            T = st("T", 3)
            C = st("C", 3)
            P = st("P", 3)
            Qm = st("Qm", 3)
            for (i, (a1, b1), (a2, b2)) in (
                (0, (qy, vz), (qz, vy)),
                (1, (qz, vx), (qx, vz)),
                (2, (qx, vy), (qy, vx)),
            ):
                nc.vector.tensor_mul(P[:, :, i:i + 1], a1, b1)
                nc.vector.tensor_mul(Qm[:, :, i:i + 1], a2, b2)
            nc.vector.scalar_tensor_tensor(
                out=T[:], in0=Qm[:], scalar=-1.0, in1=P[:],
                op0=Alu.mult, op1=Alu.add)

            # c = qv x t (into C), wt = qw*t (into WT)
            WT = st("WT", 3)
            tv = [T[:, :, i:i + 1] for i in range(3)]
            for i in range(3):
                nc.vector.tensor_mul(WT[:, :, i:i + 1], qw, tv[i])
            for (i, (a1, b1), (a2, b2)) in (
                (0, (qy, tv[2]), (qz, tv[1])),
                (1, (qz, tv[0]), (qx, tv[2])),
                (2, (qx, tv[1]), (qy, tv[0])),
            ):
                nc.vector.tensor_mul(P[:, :, i:i + 1], a1, b1)
                nc.vector.tensor_mul(Qm[:, :, i:i + 1], a2, b2)
            nc.vector.scalar_tensor_tensor(
                out=C[:], in0=Qm[:], scalar=-1.0, in1=P[:],
                op0=Alu.mult, op1=Alu.add)

            # m = c + wt (fat); e = m*g; y = v + e (fat). For the final
            # granule, per-mb slices let the last y-store DMAs overlap the
            # tail of the rotate instead of waiting for all of it.
            tail_splits = ((0, n_el),) if g0 + gn < NMB else tuple(
                (192 * kk2, 192 * (kk2 + 1)) for kk2 in range(gn))
            for lo, hi in tail_splits:
                nc.vector.tensor_add(C[:, lo:hi, :], C[:, lo:hi, :],
                                     WT[:, lo:hi, :])
                for i in range(3):
                    nc.vector.tensor_mul(C[:, lo:hi, i:i + 1],
                                         C[:, lo:hi, i:i + 1], gg[:, lo:hi, :])
                nc.vector.tensor_add(V[:, lo:hi, 0:3], C[:, lo:hi, :],
                                     V[:, lo:hi, 0:3])

            # ---- store y (in-place in vr_gran) ----
            for k in range(gn):
                nc.sync.dma_start(out=y3[g0 + k],
                                  in_=vr_gran[:, 768 * k:768 * (k + 1)])

    nc.compile()
    return nc


def _get_compiled():
    global _compiled
    if _compiled is None:
        _compiled = _build_kernel()
    return _compiled


def kernel(x, weights, VR):
    from concourse import bass_utils

    x = np.asarray(x, dtype=np.float32)
    weights = np.asarray(weights, dtype=np.float32)
    VR = np.asarray(VR, dtype=np.float32)

    qm4 = x.reshape(10, 4)
    bd = np.zeros((120, 48), np.float32)
    for s in range(12):
        bd[10 * s:10 * s + 10, 4 * s:4 * s + 4] = qm4
    ident = np.eye(128, dtype=np.float32)

    w_pad = np.zeros((N_PAD, 10), np.float32)
    w_pad[:N_NODES] = weights
    vr_pad = np.zeros(N_PAD * 4, np.float32)
    vr_pad[:N_NODES * 4] = VR

    w_shards = w_pad.reshape(N_CORES, NPC * 10)
    vr_shards = vr_pad.reshape(N_CORES, NPC * 4)

    nc = _get_compiled()
    in_maps = [
        {"w": w_shards[c], "vr": vr_shards[c], "bd": bd, "ident": ident}
        for c in range(N_CORES)
    ]
    res = bass_utils.run_bass_kernel_spmd(nc, in_maps, core_ids=list(range(N_CORES)))
    y = np.concatenate([res.results[c]["y"] for c in range(N_CORES)])
    return y[:N_NODES * 4].astype(np.float32)


if __name__ == "__main__":
    # quick self-check with random data
    rng = np.random.default_rng(0)
    x = rng.standard_normal(40).astype(np.float32)
    W = (rng.standard_normal((N_NODES, 10)) * 0.1).astype(np.float32)
    VR = rng.standard_normal(N_NODES * 4).astype(np.float32)
    y = kernel(x, weights=W, VR=VR)
    print("kernel ran, y shape", y.shape, y[:8])

